# revision 11
# baseline (speedup 1.0000x reference)
"""MoE all-to-all token dispatch kernel for 8 Trainium2 NeuronCores.

Problem: out[d, t*K+k, :] = x[t, :] if expert_mapping[expert_indices[t, k]] == d
else 0, with B=4, S=4096, H=512, K=2, 64 experts, 8 devices.

Strategy: the output's leading device axis is sharded across the 8 cores —
core d produces out[d] = [T*K, H].  Only ~1/8 of each core's output rows are
nonzero (each (t, k) slot is owned by exactly one device), so instead of
writing the dense 64 MiB slab, each core gathers just its owned token rows
from HBM into SBUF (dma_gather) and scatter-adds them into the owned slots of
the output (dma_scatter_add).  The output DRAM buffer is pre-zeroed by the
runtime (run_bass_kernel_spmd zero-fills/donates ExternalOutput buffers), so
untouched rows are already correct.

Routing metadata (which rows each core owns) is computed on the host from
expert_indices/expert_mapping and passed per-core as int16 index tensors.
Per-core counts are padded to a common multiple-of-128 maxn with all-valid
indices: padded gather slots read a zero row appended to xin (index T), and
padded scatter slots add those zeros to out row 0 — a no-op.  This keeps the
instruction stream fully static (one NEFF for all 8 cores, no runtime count
registers).
"""

import numpy as np

B, S, H, K = 4, 4096, 512, 2
T = B * S          # 16384 tokens
TK = T * K         # 32768 output rows per device
D = 8              # devices / NeuronCores
E = 64             # experts

ZROW = T           # index of the appended all-zero row in xin

# Set by test harness to collect an NTFF profile; kernel() stores the
# measured exec time (ns) here after each traced run.
TRACE = False
LAST_EXEC_NS = None
LAST_RESULTS = None

_CACHE = {}


def _wrap_idxs16(vals: np.ndarray, maxn: int, pad: int) -> np.ndarray:
    """SWDGE wrapped int16 layout: element i at [i % 16, i // 16], `pad`
    tail, replicated across the 8 partition groups (128 partitions)."""
    arr = np.full(maxn, pad, np.int16)
    arr[: len(vals)] = vals.astype(np.int16)
    w = arr.reshape(maxn // 16, 16).T          # [16, maxn/16]
    return np.ascontiguousarray(np.tile(w, (8, 1)))  # [128, maxn/16]


CH = 512           # slots per chunk (multiple of 128)


def _build_module(maxn: int):
    import concourse.bacc as bacc
    import concourse.mybir as mybir
    from concourse.library_config import mlp
    from contextlib import ExitStack

    assert maxn % CH == 0
    nb = maxn // 128
    nch = maxn // CH
    nbc = CH // 128        # data columns per chunk
    wc = CH // 16          # wrapped-idx columns per chunk

    nc = bacc.Bacc("TRN2", debug=False, num_swdge_queues=4)
    xin = nc.dram_tensor("xin", [T + 1, H], mybir.dt.float32,
                         kind="ExternalInput")
    sidx = nc.dram_tensor("sidx", [128, maxn // 16], mybir.dt.int16,
                          kind="ExternalInput")
    didx = nc.dram_tensor("didx", [128, maxn // 16], mybir.dt.int16,
                          kind="ExternalInput")
    out = nc.dram_tensor("out", [TK, H], mybir.dt.float32,
                         kind="ExternalOutput")

    with (
        nc.Block() as block,
        nc.sbuf_tensor("data", [128, nb, H], mybir.dt.float32) as data,
        nc.sbuf_tensor("sidx_sb", [128, maxn // 16], mybir.dt.int16) as sidx_sb,
        nc.sbuf_tensor("didx_sb", [128, maxn // 16], mybir.dt.int16) as didx_sb,
        nc.semaphore("io") as io,
        nc.semaphore("ssem0") as ssem0,
        nc.semaphore("ssem1") as ssem1,
        ExitStack() as stack,
    ):
        gsems = [stack.enter_context(nc.semaphore(f"g{c}"))  # noqa: ANT232
                 for c in range(nch)]

        @block.gpsimd
        def _(gpsimd):
            gpsimd.load_library(mlp)
            gpsimd.dma_start(sidx_sb[:], sidx[:]).then_inc(io, 16)
            gpsimd.dma_start(didx_sb[:], didx[:]).then_inc(io, 16)
            gpsimd.wait_ge(io, 32)
            # Enqueue every gather chunk up front (queues 0/2); the SDMA
            # engines drain them while scatters (queues 1/3) run behind.
            for c in range(nch):
                gpsimd.dma_gather(
                    data[:, c * nbc:(c + 1) * nbc, :], xin[:],
                    sidx_sb[:, c * wc:(c + 1) * wc], CH, CH, H,
                    single_packet=True, queue_num=(c % 2) * 2,
                ).then_inc(gsems[c], 16)
            for c in range(nch):
                gpsimd.wait_ge(gsems[c], 16)
                gpsimd.dma_scatter_add(
                    out[:], data[:, c * nbc:(c + 1) * nbc, :],
                    didx_sb[:, c * wc:(c + 1) * wc], CH, CH, H,
                    single_packet=False, queue_num=(c % 2) * 2 + 1,
                ).then_inc(ssem0 if c % 2 == 0 else ssem1, 16)
            gpsimd.wait_ge(ssem0, 16 * ((nch + 1) // 2))
            gpsimd.wait_ge(ssem1, 16 * (nch // 2))

    nc.compile()
    return nc


def kernel(input_tensor, expert_indices, expert_mapping):
    global LAST_EXEC_NS, LAST_RESULTS
    from concourse.bass_utils import run_bass_kernel_spmd

    x = np.zeros((T + 1, H), dtype=np.float32)
    x[:T] = np.asarray(input_tensor, dtype=np.float32).reshape(T, H)
    idx = np.asarray(expert_indices, dtype=np.int32).reshape(-1)
    emap = np.asarray(expert_mapping, dtype=np.int32)
    owner = emap[idx]                                  # [T*K], slot r = t*K+k

    dsts = [np.nonzero(owner == d)[0] for d in range(D)]
    maxn = -(-max(len(v) for v in dsts) // CH) * CH

    if maxn not in _CACHE:
        _CACHE[maxn] = _build_module(maxn)
    nc = _CACHE[maxn]

    in_maps = []
    for d in range(D):
        dst = dsts[d]
        src = dst // K
        in_maps.append({
            "xin": x,
            "sidx": _wrap_idxs16(src, maxn, pad=ZROW),
            "didx": _wrap_idxs16(dst, maxn, pad=0),
        })

    res = run_bass_kernel_spmd(nc, in_maps, list(range(D)), trace=TRACE)
    if TRACE:
        LAST_EXEC_NS = res.exec_time_ns
        LAST_RESULTS = res
    return np.stack([res.results[d]["out"] for d in range(D)], axis=0)


# revision 17
# speedup vs baseline: 1.2908x; 1.2908x over previous
"""MoE all-to-all token dispatch kernel for 8 Trainium2 NeuronCores.

Problem: out[d, t*K+k, :] = x[t, :] if expert_mapping[expert_indices[t, k]] == d
else 0, with B=4, S=4096, H=512, K=2, 64 experts, 8 devices.

Strategy: the output's leading device axis is sharded across the 8 cores —
core d produces out[d] = [T*K, H].  Only ~1/8 of each core's output rows are
nonzero (each (t, k) slot is owned by exactly one device), so instead of
writing the dense 64 MiB slab, each core gathers just its owned token rows
from HBM into SBUF (dma_gather) and writes them back to the owned slots of
the output with paged_writeback (plain-copy scatter; a CCE-based
dma_scatter_add measures ~6x slower per row because of the HBM
read-modify-write).  The output DRAM buffer is pre-zeroed by the runtime
(run_bass_kernel_spmd zero-fills/donates ExternalOutput buffers), so
untouched rows are already correct.

paged_writeback's V path writes, for token i with page ptr p and in-page
index j, a contiguous d_head row at dram element offset
(256*p + 2*j + 1) * d_head of the paged view.  Viewing the output (with one
extra leading row, real = dram[1:]) as pages of 256 rows covers all EVEN real
rows from view base 0 and all ODD real rows from view base 1.  Each chunk of
512 slots is therefore ordered [256 even-row slots | 256 odd-row slots] and
issues two writebacks.  Padded slots gather a zero row appended to xin and
are skipped by the writeback via page_ptr = -1.

Work is pipelined: gathers run on SWDGE queues 0/2, writebacks on queues 1/3,
with a ring of SBUF chunk buffers.  Routing metadata is computed on the host
from expert_indices/expert_mapping and passed per-core as index tensors; the
instruction stream is fully static (one NEFF for all 8 cores).
"""

import numpy as np

B, S, H, K = 4, 4096, 512, 2
T = B * S          # 16384 tokens
TK = T * K         # 32768 output rows per device
D = 8              # devices / NeuronCores
E = 64             # experts

ZROW = T           # index of the appended all-zero row in xin
CH = 1024          # slots per chunk (512 even + 512 odd)
HB = CH // 2       # slots per parity per chunk
WB = 128           # paged_writeback batch (>128 silently no-ops on this fw)
NBUF = 4           # chunk buffers in the SBUF ring (even: keeps each gather
                   # sem on one SWDGE queue)

TRACE = False
LAST_EXEC_NS = None
LAST_RESULTS = None

_CACHE = {}


def _wrap_idxs16(vals: np.ndarray, maxn: int, pad: int) -> np.ndarray:
    """SWDGE wrapped int16 layout: element i at [i % 16, i // 16], `pad`
    tail, replicated across the 8 partition groups (128 partitions)."""
    arr = np.full(maxn, pad, np.int16)
    arr[: len(vals)] = vals.astype(np.int16)
    w = arr.reshape(maxn // 16, 16).T          # [16, maxn/16]
    return np.ascontiguousarray(np.tile(w, (8, 1)))  # [128, maxn/16]


def _build_module(nch: int):
    from contextlib import ExitStack

    import concourse.bacc as bacc
    import concourse.mybir as mybir
    from concourse.library_config import attnmlp

    maxn = nch * CH
    nbc = CH // 128        # data columns per chunk (4)
    wc = CH // 16          # wrapped-idx16 columns per chunk (32)
    iwc = 6 * HB           # idx32 columns per chunk (two triplet blocks)

    nc = bacc.Bacc("TRN2", debug=False, num_swdge_queues=4)
    xin = nc.dram_tensor("xin", [T + 1, H], mybir.dt.float32,
                         kind="ExternalInput")
    sidx = nc.dram_tensor("sidx", [128, maxn // 16], mybir.dt.int16,
                          kind="ExternalInput")
    widx = nc.dram_tensor("widx", [128, nch * iwc], mybir.dt.int32,
                          kind="ExternalInput")
    out = nc.dram_tensor("out", [TK + 1, H], mybir.dt.float32,
                         kind="ExternalOutput")

    # paged views: [n_pages, 128, 2*d_head*page_size//128] = [128, 128, 1024]
    view_even = out[0:TK, :].rearrange("(p j t) h -> p j (t h)", j=128, t=2)
    view_odd = out[1:TK + 1, :].rearrange("(p j t) h -> p j (t h)", j=128, t=2)

    with (
        nc.Block() as block,
        nc.sbuf_tensor("data", [128, NBUF, nbc, H], mybir.dt.float32) as data,
        nc.sbuf_tensor("sidx_sb", [128, maxn // 16], mybir.dt.int16) as sidx_sb,
        nc.sbuf_tensor("widx_sb", [128, nch * iwc], mybir.dt.int32) as widx_sb,
        nc.semaphore("io") as io,
        nc.semaphore("wsem0") as wsem0,
        nc.semaphore("wsem1") as wsem1,
        ExitStack() as stack,
    ):
        gsems = [stack.enter_context(nc.semaphore(f"g{i}"))  # noqa: ANT232
                 for i in range(min(NBUF, nch))]

        @block.gpsimd
        def _(gpsimd):
            gpsimd.load_library(attnmlp)
            gpsimd.dma_start(sidx_sb[:], sidx[:]).then_inc(io, 16)
            gpsimd.dma_start(widx_sb[:], widx[:]).then_inc(io, 16)
            gpsimd.wait_ge(io, 32)

            def gather(c):
                gpsimd.dma_gather(
                    data[:, c % NBUF, :, :], xin[:],
                    sidx_sb[:, c * wc:(c + 1) * wc], CH, CH, H,
                    single_packet=False, queue_num=(c % 2) * 2,
                ).then_inc(gsems[c % NBUF], 16)

            nwb = HB // WB      # batch-128 writebacks per parity per chunk
            for c in range(min(NBUF, nch)):
                gather(c)
            for c in range(nch):
                gpsimd.wait_ge(gsems[c % NBUF], 16 * (c // NBUF + 1))
                base = c * iwc
                for w in range(nwb):
                    gpsimd.paged_writeback(
                        view_even, data[:, c % NBUF, w, :],
                        widx_sb[:, base + 3 * WB * w:base + 3 * WB * (w + 1)],
                        WB, 1, 128, H, "v", queue_num=1,
                    ).then_inc(wsem0, 16)
                    gpsimd.paged_writeback(
                        view_odd, data[:, c % NBUF, nbc // 2 + w, :],
                        widx_sb[:, base + 3 * HB + 3 * WB * w:
                                base + 3 * HB + 3 * WB * (w + 1)],
                        WB, 1, 128, H, "v", queue_num=3,
                    ).then_inc(wsem1, 16)
                if c + NBUF < nch:
                    # ring WAR: buffer (c+NBUF)%NBUF is consumed by the
                    # writebacks just issued for chunk c
                    gpsimd.wait_ge(wsem0, 16 * nwb * (c + 1))
                    gpsimd.wait_ge(wsem1, 16 * nwb * (c + 1))
                    gather(c + NBUF)
            gpsimd.wait_ge(wsem0, 16 * nwb * nch)
            gpsimd.wait_ge(wsem1, 16 * nwb * nch)

    nc.compile()
    return nc


def _prep_core(dst: np.ndarray, nch: int):
    """Per-core host prep: chunk slot ordering + gather/writeback indices."""
    maxn = nch * CH
    evens = dst[dst % 2 == 0]
    odds = dst[dst % 2 == 1]
    src16 = np.full(maxn, ZROW, np.int16)
    widx = np.empty(nch * 6 * HB, np.int32)
    for c in range(nch):
        es = evens[c * HB:(c + 1) * HB]
        os_ = odds[c * HB:(c + 1) * HB]
        base = c * CH
        src16[base:base + len(es)] = es // K
        src16[base + HB:base + HB + len(os_)] = os_ // K
        wb = c * 6 * HB
        for off, rows, sub in ((0, es, 0), (3 * HB, os_, 1)):
            m = (rows - sub) // 2
            ptr1 = np.full(HB, -1, np.int32)
            pidx = np.zeros(HB, np.int32)
            ptr1[: len(rows)] = m // 128
            pidx[: len(rows)] = m % 128
            for w in range(HB // WB):
                blk = wb + off + 3 * WB * w
                widx[blk:blk + WB] = ptr1[WB * w:WB * (w + 1)]
                widx[blk + WB:blk + 2 * WB] = -1            # ptr2: never used
                widx[blk + 2 * WB:blk + 3 * WB] = pidx[WB * w:WB * (w + 1)]
    return src16, widx


def kernel(input_tensor, expert_indices, expert_mapping):
    global LAST_EXEC_NS, LAST_RESULTS
    from concourse.bass_utils import run_bass_kernel_spmd

    x = np.zeros((T + 1, H), dtype=np.float32)
    x[:T] = np.asarray(input_tensor, dtype=np.float32).reshape(T, H)
    idx = np.asarray(expert_indices, dtype=np.int32).reshape(-1)
    emap = np.asarray(expert_mapping, dtype=np.int32)
    owner = emap[idx]                                  # [T*K], slot r = t*K+k

    dsts = [np.nonzero(owner == d)[0] for d in range(D)]
    nch = max(
        max(-(-int((dst % 2 == 0).sum()) // HB),
            -(-int((dst % 2 == 1).sum()) // HB))
        for dst in dsts
    )

    if nch not in _CACHE:
        _CACHE[nch] = _build_module(nch)
    nc = _CACHE[nch]

    in_maps = []
    for d in range(D):
        src16, widx = _prep_core(dsts[d], nch)
        in_maps.append({
            "xin": x,
            "sidx": _wrap_idxs16(src16, nch * CH, pad=ZROW),
            "widx": np.ascontiguousarray(
                np.tile(widx[None, :], (128, 1))),
        })

    res = run_bass_kernel_spmd(nc, in_maps, list(range(D)), trace=TRACE)
    if TRACE:
        LAST_EXEC_NS = res.exec_time_ns
        LAST_RESULTS = res
    return np.stack([res.results[d]["out"][1:] for d in range(D)], axis=0)
